# revision 11
# baseline (speedup 1.0000x reference)
"""AdditiveResonanceAttention kernel for 8x Trainium2 NeuronCores.

Sharding: 8 cores = (B=4) x (L/2). Core c handles batch b=c//2, query rows
[r0, r0+1024) with r0 = (c%2)*1024, all H=8 heads. Fully independent cores
(K/V projections are duplicated across the pair) -> no collectives.

Math notes:
- physics bias gamma*m_i*m_j*cos(phi_i-phi_j) is rank-2 -> folded into the
  QK^T matmul as 2 extra contraction rows (K=66), computed in bf16.
- scores are computed TRANSPOSED (ST[j,i]); softmax key-reduction is a
  ones-column in the PV stationary; PV needs no transpose.
- scores arrive in PSUM pre-scaled by C1=128/ln2 (folded into Wq and the
  physics gamma row), so softmax exponentials are produced DIRECTLY IN
  BF16 two ways, split across two engines:
    * DVE (Schraudolph): i16 = rint(max(st,-16256)+16256), bitcast to
      bf16 == 2^((i16-16256)/128) ~= e^s (piecewise-linear exp, ~3% rms)
    * ACT: exp(st/C1) with bf16 output
- QKV projections run as fp8 DoubleRow matmuls (2 contraction planes/cell
  = 2x PE throughput); weights and hidden states pre-scaled x4 on the host
  for fp8 range, descaled in the PSUM evacuations. QK^T, PV and out-proj
  stay bf16: on TRN2 the HAM clock-gate does not count DoubleRow matmuls
  as PE activity, so a DR-heavy steady-state loop gets stuck at 1.2 GHz
  (measured); bf16 keeps the PE at 2.4 GHz.
- ctx is normalized per-head (1/denominator broadcast) into bf16 ctxT;
  O-projection + residual + layernorm in bf16/fp32 as usual.
- biases bq/bk/bv/bo, mask, ln affine are all zero/identity in this
  problem; asserted on the host and folded away.
- SPMD: per-key inputs are cyclically permuted host-side so the core's
  query rows come first (softmax is invariant under key permutation).
"""
import sys

for _p in ("/opt/trn_rl_repo", "/root/.axon_site/_ro/trn_rl_repo"):
    if _p not in sys.path:
        sys.path.insert(0, _p)

import os as _os

import numpy as np
import ml_dtypes

import concourse.bass as bass
import concourse.bacc as bacc
import concourse.tile as tile
from concourse import mybir
from concourse.bass_utils import run_bass_kernel_spmd

F32 = mybir.dt.float32
BF16 = mybir.dt.bfloat16
F8 = mybir.dt.float8e4
I16 = mybir.dt.int16
AF = mybir.ActivationFunctionType
AO = mybir.AluOpType
DR = mybir.MatmulPerfMode.DoubleRow

B, L, D, H, DH = 4, 2048, 512, 8, 64
LQ = L // 2            # query rows per core
NCORES = 8
KEXT = DH + 2          # qk contraction with 2 physics rows
LN_EPS = 1e-12
NJT = L // 128         # 16 key tiles
NQB = LQ // 512        # 2 query blocks of 512
NDC = D // 128         # 4 feature chunks
NDCP = NDC // 2        # 2 feature chunk pairs (DoubleRow proj)
HALF_PI = float(np.pi / 2.0)
TWO_PI = 2.0 * float(np.pi)
C1 = 128.0 / float(np.log(2.0))        # score prescale for bf16-bitcast exp
SCL_INV = float(1.0 / C1)              # ACT exp scale
EXPB = 16256.0                         # bf16 bits of 1.0 (0x3f80)
W4 = 4.0                               # host weight prescale for fp8
SK = 1.0 / W4                          # k evac descale
SQ = (C1 / 8.0) / W4                   # q evac descale * C1/sqrt(DH)
SV = 1.0 / W4                          # v evac descale
# whole exp tiles are assigned to DVE (Schraudolph) or ACT (true exp),
# interleaved so consecutive tiles run on different engines and pipeline
EXP_DVE_N = int(_os.environ.get("KERNEL_EXP_DVE", "59"))  # of 128 tiles

_CACHED_NC = None


def _use_dve_exp(idx):
    return ((idx + 1) * EXP_DVE_N) // 128 - (idx * EXP_DVE_N) // 128 > 0


def build_nc():
    nc = bacc.Bacc()

    hp0 = nc.dram_tensor("hp0", [128, 2 * L], F8, kind="ExternalInput")
    hp1 = nc.dram_tensor("hp1", [128, 2 * L], F8, kind="ExternalInput")
    hres = nc.dram_tensor("hres", [LQ, D], F32, kind="ExternalInput")
    wkp = nc.dram_tensor("wkp", [128, NDC * D], F8, kind="ExternalInput")
    wqp = nc.dram_tensor("wqp", [128, NDC * D], F8, kind="ExternalInput")
    wvp = nc.dram_tensor("wvp", [128, NDC * D], F8, kind="ExternalInput")
    woT = nc.dram_tensor("woT", [D, D], BF16, kind="ExternalInput")
    phim = nc.dram_tensor("phim", [128, 128], F32, kind="ExternalInput")
    magm = nc.dram_tensor("magm", [128, 128], F32, kind="ExternalInput")
    phiq = nc.dram_tensor("phiq", [64, 128], F32, kind="ExternalInput")
    magq = nc.dram_tensor("magq", [64, 128], F32, kind="ExternalInput")
    gvecq = nc.dram_tensor("gvecq", [64, 1], F32, kind="ExternalInput")
    out = nc.dram_tensor("out", [LQ, D], F32, kind="ExternalOutput")

    with tile.TileContext(nc) as tc:
        _emit(nc, tc, locals())
    nc.compile()
    return nc


def _emit(nc, tc, t):
    from contextlib import ExitStack

    ts_ = bass.ts

    with ExitStack() as top:
        const = top.enter_context(tc.tile_pool(name="const", bufs=1))
        persist = top.enter_context(tc.tile_pool(name="persist", bufs=1))
        psp = top.enter_context(tc.tile_pool(name="psp", bufs=1, space="PSUM"))

        # ---- tiny constants (gpsimd queue; scalar kept clear for exp) ----
        halfpi = const.tile([128, 1], F32, tag="halfpi")
        nc.vector.memset(halfpi[:], HALF_PI)
        epst = const.tile([128, 1], F32, tag="epst")
        nc.vector.memset(epst[:], LN_EPS)
        zero_t = const.tile([128, 1], F32, tag="zero")
        nc.vector.memset(zero_t[:], 0.0)
        ones8 = const.tile([128, H], BF16, tag="ones8")
        nc.vector.memset(ones8[:], 1.0)

        # ---- persistent attention operands ----
        kT = persist.tile([KEXT, H * L], BF16, tag="kT", name="kT")
        qT = persist.tile([KEXT, H * LQ], BF16, tag="qT", name="qT")
        v_sb = [persist.tile([128, H * (DH + 1)], BF16, tag=f"v{jt}",
                             name=f"v{jt}") for jt in range(NJT)]
        ctxT = [persist.tile([128, 2, LQ], BF16, tag=f"ctxT{c}",
                             name=f"ctxT{c}") for c in range(NDCP)]

        # ---- input loads: physics first, then weights/h on sync+gpsimd ----
        hwp = top.enter_context(tc.tile_pool(name="hw", bufs=1))
        php_early = top.enter_context(tc.tile_pool(name="physin", bufs=1))
        phi_sb = php_early.tile([128, 128], F32, tag="phi")
        nc.sync.dma_start(phi_sb[:], t["phim"][:])
        mag_sb = php_early.tile([128, 128], F32, tag="mag")
        nc.gpsimd.dma_start(mag_sb[:], t["magm"][:])
        phiq_sb = php_early.tile([64, 128], F32, tag="phiq")
        nc.sync.dma_start(phiq_sb[:], t["phiq"][:])
        magq_sb = php_early.tile([64, 128], F32, tag="magq")
        nc.gpsimd.dma_start(magq_sb[:], t["magq"][:])
        gq_sb = const.tile([64, 1], F32, tag="gq")
        nc.gpsimd.dma_start(gq_sb[:], t["gvecq"][:])
        _qs = [nc.sync, nc.gpsimd]
        _qi = [0]

        def _load(dst_ap, src_ap):
            _qs[_qi[0] % len(_qs)].dma_start(dst_ap, src_ap)
            _qi[0] += 1

        # ---- weight/h loads ----
        wk_sb, wq_sb, wv_sb = ({}, {}, {})
        for nm, store in (("wkp", wk_sb), ("wqp", wq_sb), ("wvp", wv_sb)):
            for dcp in range(NDCP):
                store[dcp] = hwp.tile([128, 2, D], F8, tag=f"{nm}{dcp}",
                                      name=f"{nm}{dcp}")
        hp_sb = [hwp.tile([128, 2, L], F8, tag=f"hp{dcp}", name=f"hp{dcp}")
                 for dcp in range(NDCP)]

        # ---- physics: mag*cos(phi), mag*sin(phi) rows of kT/qT ----
        php = top.enter_context(tc.tile_pool(name="phys", bufs=1))

        def _sincos(src, p, nm):
            outs = []
            for pre, bias in ((HALF_PI, halfpi), (0.0, zero_t)):
                sfx = f"{nm}{'c' if pre else 's'}"
                tt = php.tile([p, 128], F32, tag=f"t{sfx}", name=f"t{sfx}")
                nc.vector.tensor_scalar(tt[:], src[:], 1.0 / TWO_PI,
                                        pre / TWO_PI, AO.mult, AO.add)
                ti = php.tile([p, 128], mybir.dt.int32, tag=f"ti{sfx}",
                              name=f"ti{sfx}")
                nc.vector.tensor_copy(ti[:], tt[:])
                tf = php.tile([p, 128], F32, tag=f"tf{sfx}", name=f"tf{sfx}")
                nc.vector.tensor_copy(tf[:], ti[:])
                red = php.tile([p, 128], F32, tag=f"red{sfx}", name=f"red{sfx}")
                nc.vector.scalar_tensor_tensor(red[:], tf[:], -TWO_PI,
                                               src[:], AO.mult, AO.add)
                o = php.tile([p, 128], F32, tag=f"sc{sfx}", name=f"sc{sfx}")
                nc.scalar.activation(o[:], red[:], AF.Sin, bias=bias[0:p, 0:1])
                outs.append(o)
            return outs  # [cos, sin]

        cosk, sink = _sincos(phi_sb, 128, "k")
        mc = php.tile([128, 128], BF16, tag="mc")
        nc.vector.tensor_mul(mc[:], cosk[:], mag_sb[:])
        ms = php.tile([128, 128], BF16, tag="ms")
        nc.vector.tensor_mul(ms[:], sink[:], mag_sb[:])

        cosq, sinq = _sincos(phiq_sb, 64, "q")
        gmq = php.tile([64, 128], F32, tag="gmq")
        nc.vector.tensor_scalar_mul(gmq[:], magq_sb[:], gq_sb[:, 0:1])
        mcq = php.tile([64, 128], BF16, tag="mcq")
        nc.vector.tensor_mul(mcq[:], cosq[:], gmq[:])
        msq = php.tile([64, 128], BF16, tag="msq")
        nc.vector.tensor_mul(msq[:], sinq[:], gmq[:])

        # wk first (K proj is first consumer), then hp, wq, wv
        for dcp in range(NDCP):
            _load(wk_sb[dcp][:],
                  t["wkp"][:, ts_(dcp, 1024)].rearrange("p (t c) -> p t c", t=2))
        for lb in range(4):
            for dcp in range(NDCP):
                _load(hp_sb[dcp][:, :, ts_(lb, 512)],
                      t[f"hp{dcp}"][:].rearrange("p (t l) -> p t l", t=2)
                      [:, :, ts_(lb, 512)])
        for dcp in range(NDCP):
            _load(wq_sb[dcp][:],
                  t["wqp"][:, ts_(dcp, 1024)].rearrange("p (t c) -> p t c", t=2))
        for dcp in range(NDCP):
            _load(wv_sb[dcp][:],
                  t["wvp"][:, ts_(dcp, 1024)].rearrange("p (t c) -> p t c", t=2))
        # wo + residual tiles (loads deferred into the head loop)
        wo_sb = [hwp.tile([128, D], BF16, tag=f"wo{dc}", name=f"wo{dc}")
                 for dc in range(NDC)]
        res_sb = [hwp.tile([128, D], F32, tag=f"res{lc}", name=f"res{lc}")
                  for lc in range(LQ // 128)]

        # physics rows -> kT/qT (sync HWDGE queue)
        nc.sync.dma_start(kT[64:65, :], mc[:])
        nc.sync.dma_start(kT[65:66, :], ms[:])
        nc.sync.dma_start(qT[64:65, :], mcq[:])
        nc.sync.dma_start(qT[65:66, :], msq[:])

        # ---- projections + attention, interleaved ----
        with tc.tile_pool(name="pairk", bufs=2) as pkp, \
             tc.tile_pool(name="pairq", bufs=2) as pqp, \
             tc.tile_pool(name="epool", bufs=3) as epool, \
             tc.tile_pool(name="dnp", bufs=2) as dnp, \
             tc.tile_pool(name="bcp", bufs=2) as bcp, \
             tc.tile_pool(name="cup", bufs=2) as cup:

            def kq_proj_steps(oc):
                """Emit kq projection for output-chunk oc as 16 small steps,
                interleaved into a head's jt loop to avoid bunching the
                evac work on any one engine."""
                pairk = pkp.tile([128, L], BF16, tag="pairk",
                                 name=f"pairk{oc}")
                pairq = pqp.tile([128, LQ], BF16, tag="pairq",
                                 name=f"pairq{oc}")
                ps_hold = [None]

                def step(i, evac_dve):
                    if i < 8:  # K: 4 lb blocks, (mm, evac) alternating
                        lb = i // 2
                        if i % 2 == 0:
                            ps = psp.tile([128, 512], F32, tag="proj", bufs=2,
                                          name=f"psk{oc}_{lb}")
                            for dcp in range(NDCP):
                                nc.tensor.matmul(
                                    ps[:], wk_sb[dcp][:, :, ts_(oc, 128)],
                                    hp_sb[dcp][:, :, ts_(lb, 512)],
                                    start=(dcp == 0), stop=(dcp == NDCP - 1),
                                    perf_mode=DR)
                            ps_hold[0] = ps
                        elif evac_dve:
                            nc.vector.tensor_scalar_mul(
                                pairk[:, ts_(lb, 512)], ps_hold[0][:], SK)
                        else:
                            nc.scalar.mul(pairk[:, ts_(lb, 512)],
                                          ps_hold[0][:], SK)
                    elif i < 12:  # Q: 2 qb blocks
                        qb = (i - 8) // 2
                        if i % 2 == 0:
                            ps = psp.tile([128, 512], F32, tag="proj", bufs=2,
                                          name=f"psq{oc}_{qb}")
                            for dcp in range(NDCP):
                                nc.tensor.matmul(
                                    ps[:], wq_sb[dcp][:, :, ts_(oc, 128)],
                                    hp_sb[dcp][:, :, ts_(qb, 512)],
                                    start=(dcp == 0), stop=(dcp == NDCP - 1),
                                    perf_mode=DR)
                            ps_hold[0] = ps
                        elif evac_dve:
                            nc.vector.tensor_scalar_mul(
                                pairq[:, ts_(qb, 512)], ps_hold[0][:], SQ)
                        else:
                            nc.scalar.mul(pairq[:, ts_(qb, 512)],
                                          ps_hold[0][:], SQ)
                    elif i == 12:
                        nc.gpsimd.dma_start(
                            kT[0:64, (2 * oc) * L:(2 * oc) * L + L],
                            pairk[0:64, :])
                        nc.sync.dma_start(
                            kT[0:64, (2 * oc + 1) * L:(2 * oc + 2) * L],
                            pairk[64:128, :])
                    elif i == 13:
                        nc.gpsimd.dma_start(
                            qT[0:64, (2 * oc) * LQ:(2 * oc) * LQ + LQ],
                            pairq[0:64, :])
                        nc.sync.dma_start(
                            qT[0:64, (2 * oc + 1) * LQ:(2 * oc + 2) * LQ],
                            pairq[64:128, :])
                return step

            # preload the exp ACT-table set while startup is DMA-bound
            expwarm = pkp.tile([128, 1], BF16, tag="expwarm", bufs=1)
            nc.scalar.activation(expwarm[:], zero_t[:], AF.Exp,
                                 bias=zero_t[:, 0:1], scale=SCL_INV)

            # oc=0 runs as one block before head 0 (startup is DMA-bound,
            # the engines are otherwise idle there)
            _s = kq_proj_steps(0)
            for i in range(14):
                _s(i, evac_dve=(i % 4 == 1))

            PV_LAG = 2  # PE queue is FIFO: give exps 2 QK groups of slack
            for h in range(H):
                proj_step = kq_proj_steps((h + 1) // 2) if h in (1, 3, 5) \
                    else None
                if h == 2:  # park tail-only loads once startup DMA drained
                    for dc in range(NDC):
                        nc.gpsimd.dma_start(wo_sb[dc][:],
                                            t["woT"][ts_(dc, 128), :])
                    for lc in range(LQ // 128):
                        nc.gpsimd.dma_start(res_sb[lc][:],
                                            t["hres"][ts_(lc, 128), :])
                pv = psp.tile([65, LQ], F32, tag="pv", bufs=1, name=f"pv{h}")
                epend = []

                def emit_pv(jt, e):
                    for qb in range(NQB):
                        nc.tensor.matmul(
                            pv[:, ts_(qb, 512)],
                            v_sb[jt][:, h * (DH + 1):(h + 1) * (DH + 1)],
                            e[:, ts_(qb, 512)],
                            start=(jt == 0), stop=(jt == NJT - 1))

                for jt in range(NJT):
                    if h == 0:
                        # v projection for key tile jt, just ahead of its PV
                        ps = psp.tile([128, 512], F32, tag="proj", bufs=2,
                                      name=f"psv{jt}")
                        for dcp in range(NDCP):
                            nc.tensor.matmul(
                                ps[:], hp_sb[dcp][:, :, ts_(jt, 128)],
                                wv_sb[dcp][:],
                                start=(dcp == 0), stop=(dcp == NDCP - 1),
                                perf_mode=DR)
                        vv = v_sb[jt][:].rearrange("p (h d) -> p h d", h=H)
                        if jt % 2 == 0:
                            nc.vector.tensor_scalar_mul(
                                vv[:, :, 0:DH],
                                ps[:].rearrange("p (h d) -> p h d", h=H), SV)
                        else:
                            nc.scalar.mul(
                                vv[:, :, 0:DH],
                                ps[:].rearrange("p (h d) -> p h d", h=H), SV)
                        nc.vector.tensor_copy(vv[:, :, DH:DH + 1],
                                              ones8[:, :, None])
                    if proj_step is not None and jt < 14:
                        proj_step(jt, evac_dve=(jt % 4 == 1))
                    st = psp.tile([128, LQ], F32, tag="st", bufs=2,
                                  name=f"st{h}_{jt}")
                    for qb in range(NQB):
                        nc.tensor.matmul(
                            st[:, ts_(qb, 512)],
                            kT[:, h * L + jt * 128:h * L + (jt + 1) * 128],
                            qT[:, h * LQ + qb * 512:h * LQ + (qb + 1) * 512],
                            start=True, stop=True)
                    e = epool.tile([128, LQ], BF16, tag="e", name=f"e{h}_{jt}")
                    nc.scalar.activation(e[:, 0:512], st[:, 0:512], AF.Exp,
                                         bias=zero_t[:, 0:1], scale=SCL_INV)
                    nc.vector.tensor_scalar(
                        e[:, 512:LQ].bitcast(I16), st[:, 512:LQ],
                        -EXPB, EXPB, AO.max, AO.add)
                    epend.append((jt, e))
                    if len(epend) > PV_LAG:
                        emit_pv(*epend.pop(0))
                for jt, e in epend:
                    emit_pv(jt, e)
                # evacuate pv fast (bf16), then normalize off the critical path
                cu = cup.tile([65, LQ], BF16, tag="cu", name=f"cu{h}")
                nc.scalar.copy(cu[:, 0:512], pv[:, 0:512])
                nc.vector.tensor_copy(cu[:, 512:LQ], pv[:, 512:LQ])
                dg = dnp.tile([16, 64], BF16, tag="dg", name=f"dg{h}")
                nc.sync.dma_start(dg[:], cu[64:65, :])
                dgr = dnp.tile([16, 64], BF16, tag="dgr", name=f"dgr{h}")
                with nc.allow_low_precision(
                        reason="softmax denominators need ~0.5% accuracy"):
                    nc.vector.reciprocal(dgr[:], dg[:])
                drow = dnp.tile([1, LQ], BF16, tag="drow", name=f"drow{h}")
                nc.sync.dma_start(drow[0:1, :], dgr[:])
                bc = bcp.tile([64, LQ], BF16, tag="bc", name=f"bc{h}")
                dst = ctxT[h // 4][64 * (h % 2):64 * (h % 2) + 64,
                                  (h // 2) % 2:(h // 2) % 2 + 1, :]
                dstf = dst.rearrange("p o q -> p (o q)")
                if h < H - 1:
                    # off the critical path: gpsimd does the normalize so the
                    # DVE queue stays clear for the next head's exponentials
                    nc.gpsimd.partition_broadcast(bc[:], drow[0:1, :])
                    nc.gpsimd.tensor_mul(dstf, cu[0:64, :], bc[:])
                else:
                    # last head: latency-critical chain into the out-proj
                    for qb in range(NQB):
                        nc.gpsimd.partition_broadcast(bc[:, ts_(qb, 512)],
                                                      drow[0:1, ts_(qb, 512)])
                        nc.vector.tensor_mul(dstf[:, ts_(qb, 512)],
                                             cu[0:64, ts_(qb, 512)],
                                             bc[:, ts_(qb, 512)])

        # ---- output projection + residual + layernorm ----
        for wi in range(12):
            stw = psp.tile([128, LQ], F32, tag="st", bufs=2, name=f"warm{wi}")
            for qb in range(NQB):
                nc.tensor.matmul(
                    stw[:, ts_(qb, 512)], kT[:, (wi % 8) * 128:(wi % 8 + 1) * 128],
                    qT[:, qb * 512:(qb + 1) * 512], start=True, stop=True)
        with tc.tile_pool(name="lnp", bufs=4) as lnp:
            for lc in range(LQ // 128):
                ps = psp.tile([128, D], F32, tag="proj", bufs=2,
                              name=f"pso{lc}")
                for dc in range(NDC):
                    nc.tensor.matmul(
                        ps[:],
                        ctxT[dc // 2][:, dc % 2:dc % 2 + 1,
                                      ts_(lc, 128)].rearrange(
                                          "p o q -> p (o q)"),
                        wo_sb[dc][:], start=(dc == 0), stop=(dc == NDC - 1))
                x = lnp.tile([128, D], F32, tag="x")
                nc.vector.tensor_add(x[:], ps[:], res_sb[lc][:])
                stats = lnp.tile([128, 6], F32, tag="stats")
                nc.vector.bn_stats(stats[:], x[:])
                mv = lnp.tile([128, 2], F32, tag="mv")
                nc.vector.bn_aggr(mv[:], stats[:])
                sd = lnp.tile([128, 1], F32, tag="sd")
                nc.scalar.activation(sd[:], mv[:, 1:2], AF.Sqrt,
                                     bias=epst[:, 0:1])
                rstd = lnp.tile([128, 1], F32, tag="rstd")
                nc.vector.reciprocal(rstd[:], sd[:])
                nmr = lnp.tile([128, 1], F32, tag="nmr")
                nc.vector.scalar_tensor_tensor(
                    nmr[:], mv[:, 0:1], -1.0, rstd[:, 0:1], AO.mult, AO.mult)
                o = lnp.tile([128, D], F32, tag="oo")
                if lc % 2 == 0:
                    nc.scalar.activation(o[:], x[:], AF.Identity,
                                         bias=nmr[:, 0:1], scale=rstd[:, 0:1])
                else:
                    nc.vector.tensor_scalar(o[:], x[:], rstd[:, 0:1],
                                            nmr[:, 0:1], AO.mult, AO.add)
                nc.sync.dma_start(t["out"][ts_(lc, 128), :], o[:])


def _host_prep(inputs):
    np_f8 = ml_dtypes.float8_e4m3
    hs = np.ascontiguousarray(np.asarray(inputs["hidden_states"],
                                         dtype=np.float32))
    am = np.asarray(inputs["attention_mask"], dtype=np.float32)
    phi = np.asarray(inputs["phi"], dtype=np.float32)
    mag = np.asarray(inputs["mag"], dtype=np.float32)
    Wq = np.asarray(inputs["Wq"], dtype=np.float32)
    Wk = np.asarray(inputs["Wk"], dtype=np.float32)
    Wv = np.asarray(inputs["Wv"], dtype=np.float32)
    Wo = np.asarray(inputs["Wo"], dtype=np.float32)
    gamma = np.asarray(inputs["gamma"], dtype=np.float32).reshape(H)
    ln_w = np.asarray(inputs["ln_w"], dtype=np.float32)
    ln_b = np.asarray(inputs["ln_b"], dtype=np.float32)
    for nm in ("bq", "bk", "bv", "bo"):
        assert not np.any(np.asarray(inputs[nm])), \
            f"kernel assumes zero {nm}; generalize if this fires"
    assert not np.any(am), "kernel assumes zero attention mask"
    assert np.allclose(ln_w, 1.0) and np.allclose(ln_b, 0.0), \
        "kernel folds ln affine away; generalize if this fires"

    def packw(W):
        w4 = np.clip(np.ascontiguousarray(W.T) * W4, -240, 240)
        return np.ascontiguousarray(
            w4.reshape(NDC, 128, D).transpose(1, 0, 2).reshape(128, NDC * D)
        ).astype(np_f8)

    wkp = packw(Wk)
    wqp = packw(Wq)
    wvp = packw(Wv)
    woT = np.ascontiguousarray(Wo.T).astype(ml_dtypes.bfloat16)
    gvecq = (np.repeat(gamma, 8)[:, None] * C1).astype(np.float32)

    in_maps = []
    for c in range(NCORES):
        b, half = c // 2, c % 2
        r0 = half * LQ
        perm = np.roll(np.arange(L), -r0)  # query half first
        hTb = np.ascontiguousarray(hs[b].T[:, perm])
        hp = np.clip(hTb, -240, 240).reshape(NDC, 128, L)
        hp0 = np.ascontiguousarray(
            hp[0:2].transpose(1, 0, 2).reshape(128, 2 * L)).astype(np_f8)
        hp1 = np.ascontiguousarray(
            hp[2:4].transpose(1, 0, 2).reshape(128, 2 * L)).astype(np_f8)
        phip = np.ascontiguousarray(phi[b][:, perm])
        magp = np.ascontiguousarray(mag[b][:, perm])
        hresb = np.ascontiguousarray(hs[b, r0:r0 + LQ])
        in_maps.append(dict(
            hp0=hp0, hp1=hp1, hres=hresb,
            wkp=wkp, wqp=wqp, wvp=wvp, woT=woT,
            phim=phip.reshape(128, 128), magm=magp.reshape(128, 128),
            phiq=np.ascontiguousarray(phip[:, :LQ]).reshape(64, 128),
            magq=np.ascontiguousarray(magp[:, :LQ]).reshape(64, 128),
            gvecq=gvecq,
        ))
    return in_maps


def _get_nc():
    global _CACHED_NC
    if _CACHED_NC is None:
        _CACHED_NC = build_nc()
    return _CACHED_NC


def run(inputs, **spmd_kwargs):
    in_maps = _host_prep(inputs)
    nc = _get_nc()
    res = run_bass_kernel_spmd(nc, in_maps, core_ids=list(range(NCORES)),
                               **spmd_kwargs)
    out = np.empty((B, L, D), dtype=np.float32)
    for c in range(NCORES):
        b, half = c // 2, c % 2
        out[b, half * LQ:(half + 1) * LQ] = res.results[c]["out"]
    return out, res


def kernel(**inputs) -> np.ndarray:
    out, _ = run(inputs)
    return out


# revision 12
# speedup vs baseline: 1.1134x; 1.1134x over previous
"""AdditiveResonanceAttention kernel for 8x Trainium2 NeuronCores.

Sharding: 8 cores = (B=4) x (L/2). Core c handles batch b=c//2, query rows
[r0, r0+1024) with r0 = (c%2)*1024, all H=8 heads. Fully independent cores
(K/V projections are duplicated across the pair) -> no collectives.

Math notes:
- physics bias gamma*m_i*m_j*cos(phi_i-phi_j) is rank-2 -> folded into the
  QK^T matmul as 2 extra contraction rows (K=66), computed in bf16.
- scores are computed TRANSPOSED (ST[j,i]); softmax key-reduction is a
  ones-column in the PV stationary; PV needs no transpose.
- scores arrive in PSUM pre-scaled by C1=128/ln2 (folded into Wq and the
  physics gamma row), so softmax exponentials are produced DIRECTLY IN
  BF16 two ways, split across two engines:
    * DVE (Schraudolph): i16 = rint(max(st,-16256)+16256), bitcast to
      bf16 == 2^((i16-16256)/128) ~= e^s (piecewise-linear exp, ~3% rms)
    * ACT: exp(st/C1) with bf16 output
- QKV projections run as fp8 DoubleRow matmuls (2 contraction planes/cell
  = 2x PE throughput); weights and hidden states pre-scaled x4 on the host
  for fp8 range, descaled in the PSUM evacuations. QK^T, PV and out-proj
  stay bf16: on TRN2 the HAM clock-gate does not count DoubleRow matmuls
  as PE activity, so a DR-heavy steady-state loop gets stuck at 1.2 GHz
  (measured); bf16 keeps the PE at 2.4 GHz.
- ctx is normalized per-head (1/denominator broadcast) into bf16 ctxT;
  O-projection + residual + layernorm in bf16/fp32 as usual.
- biases bq/bk/bv/bo, mask, ln affine are all zero/identity in this
  problem; asserted on the host and folded away.
- SPMD: per-key inputs are cyclically permuted host-side so the core's
  query rows come first (softmax is invariant under key permutation).
"""
import sys

for _p in ("/opt/trn_rl_repo", "/root/.axon_site/_ro/trn_rl_repo"):
    if _p not in sys.path:
        sys.path.insert(0, _p)

import os as _os

import numpy as np
import ml_dtypes

import concourse.bass as bass
import concourse.bacc as bacc
import concourse.tile as tile
from concourse import mybir
from concourse.bass_utils import run_bass_kernel_spmd

F32 = mybir.dt.float32
BF16 = mybir.dt.bfloat16
F8 = mybir.dt.float8e4
I16 = mybir.dt.int16
AF = mybir.ActivationFunctionType
AO = mybir.AluOpType
DR = mybir.MatmulPerfMode.DoubleRow

B, L, D, H, DH = 4, 2048, 512, 8, 64
LQ = L // 2            # query rows per core
NCORES = 8
KEXT = DH + 2          # qk contraction with 2 physics rows
LN_EPS = 1e-12
NJT = L // 128         # 16 key tiles
NQB = LQ // 512        # 2 query blocks of 512
NDC = D // 128         # 4 feature chunks
NDCP = NDC // 2        # 2 feature chunk pairs (DoubleRow proj)
HALF_PI = float(np.pi / 2.0)
TWO_PI = 2.0 * float(np.pi)
C1 = 128.0 / float(np.log(2.0))        # score prescale for bf16-bitcast exp
SCL_INV = float(1.0 / C1)              # ACT exp scale
EXPB = 16256.0                         # bf16 bits of 1.0 (0x3f80)
W4 = 4.0                               # host weight prescale for fp8
SK = 1.0 / W4                          # k evac descale
SQ = (C1 / 8.0) / W4                   # q evac descale * C1/sqrt(DH)
SV = 1.0 / W4                          # v evac descale
# whole exp tiles are assigned to DVE (Schraudolph) or ACT (true exp),
# interleaved so consecutive tiles run on different engines and pipeline
EXP_DVE_N = int(_os.environ.get("KERNEL_EXP_DVE", "59"))  # of 128 tiles

_CACHED_NC = None


def _use_dve_exp(idx):
    return ((idx + 1) * EXP_DVE_N) // 128 - (idx * EXP_DVE_N) // 128 > 0


def build_nc():
    nc = bacc.Bacc()

    hp0 = nc.dram_tensor("hp0", [128, 2 * L], F8, kind="ExternalInput")
    hp1 = nc.dram_tensor("hp1", [128, 2 * L], F8, kind="ExternalInput")
    hres = nc.dram_tensor("hres", [LQ, D], F32, kind="ExternalInput")
    wkp = nc.dram_tensor("wkp", [128, NDC * D], F8, kind="ExternalInput")
    wqp = nc.dram_tensor("wqp", [128, NDC * D], F8, kind="ExternalInput")
    wvp = nc.dram_tensor("wvp", [128, NDC * D], F8, kind="ExternalInput")
    woT = nc.dram_tensor("woT", [D, D], BF16, kind="ExternalInput")
    phim = nc.dram_tensor("phim", [128, 128], F32, kind="ExternalInput")
    magm = nc.dram_tensor("magm", [128, 128], F32, kind="ExternalInput")
    phiq = nc.dram_tensor("phiq", [64, 128], F32, kind="ExternalInput")
    magq = nc.dram_tensor("magq", [64, 128], F32, kind="ExternalInput")
    gvecq = nc.dram_tensor("gvecq", [64, 1], F32, kind="ExternalInput")
    out = nc.dram_tensor("out", [LQ, D], F32, kind="ExternalOutput")

    with tile.TileContext(nc) as tc:
        _emit(nc, tc, locals())
    nc.compile()
    return nc


def _emit(nc, tc, t):
    from contextlib import ExitStack

    ts_ = bass.ts

    with ExitStack() as top:
        const = top.enter_context(tc.tile_pool(name="const", bufs=1))
        persist = top.enter_context(tc.tile_pool(name="persist", bufs=1))
        psp = top.enter_context(tc.tile_pool(name="psp", bufs=1, space="PSUM"))

        # ---- tiny constants (gpsimd queue; scalar kept clear for exp) ----
        halfpi = const.tile([128, 1], F32, tag="halfpi")
        nc.vector.memset(halfpi[:], HALF_PI)
        epst = const.tile([128, 1], F32, tag="epst")
        nc.vector.memset(epst[:], LN_EPS)
        zero_t = const.tile([128, 1], F32, tag="zero")
        nc.vector.memset(zero_t[:], 0.0)
        ones8 = const.tile([128, H], BF16, tag="ones8")
        nc.vector.memset(ones8[:], 1.0)

        # ---- persistent attention operands ----
        kT = persist.tile([KEXT, H * L], BF16, tag="kT", name="kT")
        qT = persist.tile([KEXT, H * LQ], BF16, tag="qT", name="qT")
        v_sb = [persist.tile([128, H * (DH + 1)], BF16, tag=f"v{jt}",
                             name=f"v{jt}") for jt in range(NJT)]
        ctxT = [persist.tile([128, 2, LQ], BF16, tag=f"ctxT{c}",
                             name=f"ctxT{c}") for c in range(NDCP)]

        # ---- input loads: physics first, then weights/h on sync+gpsimd ----
        hwp = top.enter_context(tc.tile_pool(name="hw", bufs=1))
        php_early = top.enter_context(tc.tile_pool(name="physin", bufs=1))
        phi_sb = php_early.tile([128, 128], F32, tag="phi")
        nc.sync.dma_start(phi_sb[:], t["phim"][:])
        mag_sb = php_early.tile([128, 128], F32, tag="mag")
        nc.gpsimd.dma_start(mag_sb[:], t["magm"][:])
        phiq_sb = php_early.tile([64, 128], F32, tag="phiq")
        nc.sync.dma_start(phiq_sb[:], t["phiq"][:])
        magq_sb = php_early.tile([64, 128], F32, tag="magq")
        nc.gpsimd.dma_start(magq_sb[:], t["magq"][:])
        gq_sb = const.tile([64, 1], F32, tag="gq")
        nc.gpsimd.dma_start(gq_sb[:], t["gvecq"][:])
        _qs = [nc.sync, nc.gpsimd]
        _qi = [0]

        def _load(dst_ap, src_ap):
            _qs[_qi[0] % len(_qs)].dma_start(dst_ap, src_ap)
            _qi[0] += 1

        # ---- weight/h loads ----
        wk_sb, wq_sb, wv_sb = ({}, {}, {})
        for nm, store in (("wkp", wk_sb), ("wqp", wq_sb), ("wvp", wv_sb)):
            for dcp in range(NDCP):
                store[dcp] = hwp.tile([128, 2, D], F8, tag=f"{nm}{dcp}",
                                      name=f"{nm}{dcp}")
        hp_sb = [hwp.tile([128, 2, L], F8, tag=f"hp{dcp}", name=f"hp{dcp}")
                 for dcp in range(NDCP)]

        # ---- physics: mag*cos(phi), mag*sin(phi) rows of kT/qT ----
        php = top.enter_context(tc.tile_pool(name="phys", bufs=1))

        def _sincos(src, p, nm):
            outs = []
            for pre, bias in ((HALF_PI, halfpi), (0.0, zero_t)):
                sfx = f"{nm}{'c' if pre else 's'}"
                tt = php.tile([p, 128], F32, tag=f"t{sfx}", name=f"t{sfx}")
                nc.vector.tensor_scalar(tt[:], src[:], 1.0 / TWO_PI,
                                        pre / TWO_PI, AO.mult, AO.add)
                ti = php.tile([p, 128], mybir.dt.int32, tag=f"ti{sfx}",
                              name=f"ti{sfx}")
                nc.vector.tensor_copy(ti[:], tt[:])
                tf = php.tile([p, 128], F32, tag=f"tf{sfx}", name=f"tf{sfx}")
                nc.vector.tensor_copy(tf[:], ti[:])
                red = php.tile([p, 128], F32, tag=f"red{sfx}", name=f"red{sfx}")
                nc.vector.scalar_tensor_tensor(red[:], tf[:], -TWO_PI,
                                               src[:], AO.mult, AO.add)
                o = php.tile([p, 128], F32, tag=f"sc{sfx}", name=f"sc{sfx}")
                nc.scalar.activation(o[:], red[:], AF.Sin, bias=bias[0:p, 0:1])
                outs.append(o)
            return outs  # [cos, sin]

        cosk, sink = _sincos(phi_sb, 128, "k")
        mc = php.tile([128, 128], BF16, tag="mc")
        nc.vector.tensor_mul(mc[:], cosk[:], mag_sb[:])
        ms = php.tile([128, 128], BF16, tag="ms")
        nc.vector.tensor_mul(ms[:], sink[:], mag_sb[:])

        cosq, sinq = _sincos(phiq_sb, 64, "q")
        gmq = php.tile([64, 128], F32, tag="gmq")
        nc.vector.tensor_scalar_mul(gmq[:], magq_sb[:], gq_sb[:, 0:1])
        mcq = php.tile([64, 128], BF16, tag="mcq")
        nc.vector.tensor_mul(mcq[:], cosq[:], gmq[:])
        msq = php.tile([64, 128], BF16, tag="msq")
        nc.vector.tensor_mul(msq[:], sinq[:], gmq[:])

        # wk first (K proj is first consumer), then hp, wq, wv
        for dcp in range(NDCP):
            _load(wk_sb[dcp][:],
                  t["wkp"][:, ts_(dcp, 1024)].rearrange("p (t c) -> p t c", t=2))
        for lb in range(4):
            for dcp in range(NDCP):
                _load(hp_sb[dcp][:, :, ts_(lb, 512)],
                      t[f"hp{dcp}"][:].rearrange("p (t l) -> p t l", t=2)
                      [:, :, ts_(lb, 512)])
        for dcp in range(NDCP):
            _load(wq_sb[dcp][:],
                  t["wqp"][:, ts_(dcp, 1024)].rearrange("p (t c) -> p t c", t=2))
        for dcp in range(NDCP):
            _load(wv_sb[dcp][:],
                  t["wvp"][:, ts_(dcp, 1024)].rearrange("p (t c) -> p t c", t=2))
        # wo + residual tiles (loads deferred into the head loop)
        wo_sb = [hwp.tile([128, D], BF16, tag=f"wo{dc}", name=f"wo{dc}")
                 for dc in range(NDC)]
        res_sb = [hwp.tile([128, D], F32, tag=f"res{lc}", name=f"res{lc}")
                  for lc in range(LQ // 128)]

        # physics rows -> kT/qT (sync HWDGE queue)
        nc.sync.dma_start(kT[64:65, :], mc[:])
        nc.sync.dma_start(kT[65:66, :], ms[:])
        nc.sync.dma_start(qT[64:65, :], mcq[:])
        nc.sync.dma_start(qT[65:66, :], msq[:])

        # ---- projections + attention, interleaved ----
        with tc.tile_pool(name="pairk", bufs=2) as pkp, \
             tc.tile_pool(name="pairq", bufs=2) as pqp, \
             tc.tile_pool(name="epool", bufs=3) as epool, \
             tc.tile_pool(name="dnp", bufs=2) as dnp, \
             tc.tile_pool(name="bcp", bufs=2) as bcp, \
             tc.tile_pool(name="cup", bufs=2) as cup:

            def kq_proj_steps(oc):
                """Emit kq projection for output-chunk oc as 16 small steps,
                interleaved into a head's jt loop to avoid bunching the
                evac work on any one engine."""
                pairk = pkp.tile([128, L], BF16, tag="pairk",
                                 name=f"pairk{oc}")
                pairq = pqp.tile([128, LQ], BF16, tag="pairq",
                                 name=f"pairq{oc}")
                ps_hold = [None]

                def step(i, evac_dve):
                    if i < 8:  # K: 4 lb blocks, (mm, evac) alternating
                        lb = i // 2
                        if i % 2 == 0:
                            ps = psp.tile([128, 512], F32, tag="proj", bufs=2,
                                          name=f"psk{oc}_{lb}")
                            for dcp in range(NDCP):
                                nc.tensor.matmul(
                                    ps[:], wk_sb[dcp][:, :, ts_(oc, 128)],
                                    hp_sb[dcp][:, :, ts_(lb, 512)],
                                    start=(dcp == 0), stop=(dcp == NDCP - 1),
                                    perf_mode=DR)
                            ps_hold[0] = ps
                        elif evac_dve:
                            nc.vector.tensor_scalar_mul(
                                pairk[:, ts_(lb, 512)], ps_hold[0][:], SK)
                        else:
                            nc.scalar.mul(pairk[:, ts_(lb, 512)],
                                          ps_hold[0][:], SK)
                    elif i < 12:  # Q: 2 qb blocks
                        qb = (i - 8) // 2
                        if i % 2 == 0:
                            ps = psp.tile([128, 512], F32, tag="proj", bufs=2,
                                          name=f"psq{oc}_{qb}")
                            for dcp in range(NDCP):
                                nc.tensor.matmul(
                                    ps[:], wq_sb[dcp][:, :, ts_(oc, 128)],
                                    hp_sb[dcp][:, :, ts_(qb, 512)],
                                    start=(dcp == 0), stop=(dcp == NDCP - 1),
                                    perf_mode=DR)
                            ps_hold[0] = ps
                        elif evac_dve:
                            nc.vector.tensor_scalar_mul(
                                pairq[:, ts_(qb, 512)], ps_hold[0][:], SQ)
                        else:
                            nc.scalar.mul(pairq[:, ts_(qb, 512)],
                                          ps_hold[0][:], SQ)
                    elif i == 12:
                        nc.gpsimd.dma_start(
                            kT[0:64, (2 * oc) * L:(2 * oc) * L + L],
                            pairk[0:64, :])
                        nc.sync.dma_start(
                            kT[0:64, (2 * oc + 1) * L:(2 * oc + 2) * L],
                            pairk[64:128, :])
                    elif i == 13:
                        nc.gpsimd.dma_start(
                            qT[0:64, (2 * oc) * LQ:(2 * oc) * LQ + LQ],
                            pairq[0:64, :])
                        nc.sync.dma_start(
                            qT[0:64, (2 * oc + 1) * LQ:(2 * oc + 2) * LQ],
                            pairq[64:128, :])
                return step

            # preload the exp ACT-table set while startup is DMA-bound
            expwarm = pkp.tile([128, 1], BF16, tag="expwarm", bufs=1)
            nc.scalar.activation(expwarm[:], zero_t[:], AF.Exp,
                                 bias=zero_t[:, 0:1], scale=SCL_INV)

            # oc=0 runs as one block before head 0 (startup is DMA-bound,
            # the engines are otherwise idle there)
            _s = kq_proj_steps(0)
            for i in range(14):
                _s(i, evac_dve=(i % 4 == 1))

            PV_LAG = 2  # PE queue is FIFO: give exps 2 QK groups of slack
            for h in range(H):
                proj_step = kq_proj_steps((h + 1) // 2) if h in (1, 3, 5) \
                    else None
                if h == 2:  # park tail-only loads once startup DMA drained
                    for dc in range(NDC):
                        nc.sync.dma_start(wo_sb[dc][:],
                                          t["woT"][ts_(dc, 128), :])
                elif h == 4:
                    for lc in range(4):
                        nc.gpsimd.dma_start(res_sb[lc][:],
                                            t["hres"][ts_(lc, 128), :])
                elif h == 6:
                    for lc in range(4, LQ // 128):
                        nc.sync.dma_start(res_sb[lc][:],
                                          t["hres"][ts_(lc, 128), :])
                pv = psp.tile([65, LQ], F32, tag="pv", bufs=1, name=f"pv{h}")
                epend = []

                def emit_pv(jt, e):
                    for qb in range(NQB):
                        nc.tensor.matmul(
                            pv[:, ts_(qb, 512)],
                            v_sb[jt][:, h * (DH + 1):(h + 1) * (DH + 1)],
                            e[:, ts_(qb, 512)],
                            start=(jt == 0), stop=(jt == NJT - 1))

                for jt in range(NJT):
                    if h == 0:
                        # v projection for key tile jt, just ahead of its PV
                        ps = psp.tile([128, 512], F32, tag="proj", bufs=2,
                                      name=f"psv{jt}")
                        for dcp in range(NDCP):
                            nc.tensor.matmul(
                                ps[:], hp_sb[dcp][:, :, ts_(jt, 128)],
                                wv_sb[dcp][:],
                                start=(dcp == 0), stop=(dcp == NDCP - 1),
                                perf_mode=DR)
                        vv = v_sb[jt][:].rearrange("p (h d) -> p h d", h=H)
                        if jt % 2 == 0:
                            nc.vector.tensor_scalar_mul(
                                vv[:, :, 0:DH],
                                ps[:].rearrange("p (h d) -> p h d", h=H), SV)
                        else:
                            nc.scalar.mul(
                                vv[:, :, 0:DH],
                                ps[:].rearrange("p (h d) -> p h d", h=H), SV)
                        nc.vector.tensor_copy(vv[:, :, DH:DH + 1],
                                              ones8[:, :, None])
                    if proj_step is not None and jt < 14:
                        proj_step(jt, evac_dve=(jt % 4 == 1))
                    st = psp.tile([128, LQ], F32, tag="st", bufs=2,
                                  name=f"st{h}_{jt}")
                    for qb in range(NQB):
                        nc.tensor.matmul(
                            st[:, ts_(qb, 512)],
                            kT[:, h * L + jt * 128:h * L + (jt + 1) * 128],
                            qT[:, h * LQ + qb * 512:h * LQ + (qb + 1) * 512],
                            start=True, stop=True)
                    e = epool.tile([128, LQ], BF16, tag="e", name=f"e{h}_{jt}")
                    nc.scalar.activation(e[:, 0:512], st[:, 0:512], AF.Exp,
                                         bias=zero_t[:, 0:1], scale=SCL_INV)
                    nc.vector.tensor_scalar(
                        e[:, 512:LQ].bitcast(I16), st[:, 512:LQ],
                        -EXPB, EXPB, AO.max, AO.add)
                    epend.append((jt, e))
                    if len(epend) > PV_LAG:
                        emit_pv(*epend.pop(0))
                for jt, e in epend:
                    emit_pv(jt, e)
                # evacuate pv fast (bf16), then normalize off the critical path
                cu = cup.tile([65, LQ], BF16, tag="cu", name=f"cu{h}")
                nc.scalar.copy(cu[:, 0:512], pv[:, 0:512])
                nc.vector.tensor_copy(cu[:, 512:LQ], pv[:, 512:LQ])
                dg = dnp.tile([16, 64], BF16, tag="dg", name=f"dg{h}")
                nc.sync.dma_start(dg[:], cu[64:65, :])
                dgr = dnp.tile([16, 64], BF16, tag="dgr", name=f"dgr{h}")
                with nc.allow_low_precision(
                        reason="softmax denominators need ~0.5% accuracy"):
                    nc.vector.reciprocal(dgr[:], dg[:])
                drow = dnp.tile([1, LQ], BF16, tag="drow", name=f"drow{h}")
                nc.sync.dma_start(drow[0:1, :], dgr[:])
                bc = bcp.tile([64, LQ], BF16, tag="bc", name=f"bc{h}")
                dst = ctxT[h // 4][64 * (h % 2):64 * (h % 2) + 64,
                                  (h // 2) % 2:(h // 2) % 2 + 1, :]
                dstf = dst.rearrange("p o q -> p (o q)")
                if h < H - 1:
                    nc.gpsimd.partition_broadcast(bc[:], drow[0:1, :])
                    nc.vector.tensor_mul(dstf, cu[0:64, :], bc[:])
                else:
                    # last head: latency-critical chain into the out-proj
                    for qb in range(NQB):
                        nc.gpsimd.partition_broadcast(bc[:, ts_(qb, 512)],
                                                      drow[0:1, ts_(qb, 512)])
                        nc.vector.tensor_mul(dstf[:, ts_(qb, 512)],
                                             cu[0:64, ts_(qb, 512)],
                                             bc[:, ts_(qb, 512)])

        # ---- output projection + residual + layernorm ----
        for wi in range(12):
            stw = psp.tile([128, LQ], F32, tag="st", bufs=2, name=f"warm{wi}")
            for qb in range(NQB):
                nc.tensor.matmul(
                    stw[:, ts_(qb, 512)], kT[:, (wi % 8) * 128:(wi % 8 + 1) * 128],
                    qT[:, qb * 512:(qb + 1) * 512], start=True, stop=True)
        with tc.tile_pool(name="lnp", bufs=4) as lnp:
            for lc in range(LQ // 128):
                ps = psp.tile([128, D], F32, tag="proj", bufs=2,
                              name=f"pso{lc}")
                for dc in range(NDC):
                    nc.tensor.matmul(
                        ps[:],
                        ctxT[dc // 2][:, dc % 2:dc % 2 + 1,
                                      ts_(lc, 128)].rearrange(
                                          "p o q -> p (o q)"),
                        wo_sb[dc][:], start=(dc == 0), stop=(dc == NDC - 1))
                x = lnp.tile([128, D], F32, tag="x")
                nc.vector.tensor_add(x[:], ps[:], res_sb[lc][:])
                stats = lnp.tile([128, 6], F32, tag="stats")
                nc.vector.bn_stats(stats[:], x[:])
                mv = lnp.tile([128, 2], F32, tag="mv")
                nc.vector.bn_aggr(mv[:], stats[:])
                sd = lnp.tile([128, 1], F32, tag="sd")
                nc.scalar.activation(sd[:], mv[:, 1:2], AF.Sqrt,
                                     bias=epst[:, 0:1])
                rstd = lnp.tile([128, 1], F32, tag="rstd")
                nc.vector.reciprocal(rstd[:], sd[:])
                nmr = lnp.tile([128, 1], F32, tag="nmr")
                nc.vector.scalar_tensor_tensor(
                    nmr[:], mv[:, 0:1], -1.0, rstd[:, 0:1], AO.mult, AO.mult)
                o = lnp.tile([128, D], F32, tag="oo")
                if lc % 2 == 0:
                    nc.scalar.activation(o[:], x[:], AF.Identity,
                                         bias=nmr[:, 0:1], scale=rstd[:, 0:1])
                else:
                    nc.vector.tensor_scalar(o[:], x[:], rstd[:, 0:1],
                                            nmr[:, 0:1], AO.mult, AO.add)
                nc.sync.dma_start(t["out"][ts_(lc, 128), :], o[:])


def _host_prep(inputs):
    np_f8 = ml_dtypes.float8_e4m3
    hs = np.ascontiguousarray(np.asarray(inputs["hidden_states"],
                                         dtype=np.float32))
    am = np.asarray(inputs["attention_mask"], dtype=np.float32)
    phi = np.asarray(inputs["phi"], dtype=np.float32)
    mag = np.asarray(inputs["mag"], dtype=np.float32)
    Wq = np.asarray(inputs["Wq"], dtype=np.float32)
    Wk = np.asarray(inputs["Wk"], dtype=np.float32)
    Wv = np.asarray(inputs["Wv"], dtype=np.float32)
    Wo = np.asarray(inputs["Wo"], dtype=np.float32)
    gamma = np.asarray(inputs["gamma"], dtype=np.float32).reshape(H)
    ln_w = np.asarray(inputs["ln_w"], dtype=np.float32)
    ln_b = np.asarray(inputs["ln_b"], dtype=np.float32)
    for nm in ("bq", "bk", "bv", "bo"):
        assert not np.any(np.asarray(inputs[nm])), \
            f"kernel assumes zero {nm}; generalize if this fires"
    assert not np.any(am), "kernel assumes zero attention mask"
    assert np.allclose(ln_w, 1.0) and np.allclose(ln_b, 0.0), \
        "kernel folds ln affine away; generalize if this fires"

    def packw(W):
        w4 = np.clip(np.ascontiguousarray(W.T) * W4, -240, 240)
        return np.ascontiguousarray(
            w4.reshape(NDC, 128, D).transpose(1, 0, 2).reshape(128, NDC * D)
        ).astype(np_f8)

    wkp = packw(Wk)
    wqp = packw(Wq)
    wvp = packw(Wv)
    woT = np.ascontiguousarray(Wo.T).astype(ml_dtypes.bfloat16)
    gvecq = (np.repeat(gamma, 8)[:, None] * C1).astype(np.float32)

    in_maps = []
    for c in range(NCORES):
        b, half = c // 2, c % 2
        r0 = half * LQ
        perm = np.roll(np.arange(L), -r0)  # query half first
        hTb = np.ascontiguousarray(hs[b].T[:, perm])
        hp = np.clip(hTb, -240, 240).reshape(NDC, 128, L)
        hp0 = np.ascontiguousarray(
            hp[0:2].transpose(1, 0, 2).reshape(128, 2 * L)).astype(np_f8)
        hp1 = np.ascontiguousarray(
            hp[2:4].transpose(1, 0, 2).reshape(128, 2 * L)).astype(np_f8)
        phip = np.ascontiguousarray(phi[b][:, perm])
        magp = np.ascontiguousarray(mag[b][:, perm])
        hresb = np.ascontiguousarray(hs[b, r0:r0 + LQ])
        in_maps.append(dict(
            hp0=hp0, hp1=hp1, hres=hresb,
            wkp=wkp, wqp=wqp, wvp=wvp, woT=woT,
            phim=phip.reshape(128, 128), magm=magp.reshape(128, 128),
            phiq=np.ascontiguousarray(phip[:, :LQ]).reshape(64, 128),
            magq=np.ascontiguousarray(magp[:, :LQ]).reshape(64, 128),
            gvecq=gvecq,
        ))
    return in_maps


def _get_nc():
    global _CACHED_NC
    if _CACHED_NC is None:
        _CACHED_NC = build_nc()
    return _CACHED_NC


def run(inputs, **spmd_kwargs):
    in_maps = _host_prep(inputs)
    nc = _get_nc()
    res = run_bass_kernel_spmd(nc, in_maps, core_ids=list(range(NCORES)),
                               **spmd_kwargs)
    out = np.empty((B, L, D), dtype=np.float32)
    for c in range(NCORES):
        b, half = c // 2, c % 2
        out[b, half * LQ:(half + 1) * LQ] = res.results[c]["out"]
    return out, res


def kernel(**inputs) -> np.ndarray:
    out, _ = run(inputs)
    return out
